# revision 57
# baseline (speedup 1.0000x reference)
"""Trainium2 Bass kernel for the AdSBHNet holographic-potential problem.

Key idea: all three integrands are analytic on y in [0,1] (the apparent
sqrt singularities at the endpoints cancel), so a 16-node Gauss-Legendre
rule reproduces the reference's 1000-point trapezoid to ~2.6e-5 relative
(the reference's own discretization error) -- measured in float64 against
the jax reference. That shrinks the quadrature grid 62x vs the trapz
baseline.

Sharding: data-parallel over zs across 8 NeuronCores (1024 each). Per
core the grid is [128 partitions = 8 zs-groups x 16 y-nodes, 128 free =
zs within group]. Polynomial grids (gn, fz, t1, gdd'', Pt') are built by
fp32 TensorEngine matmuls (full precision via the LOW/HIGH 2-pass) with
block-diagonal per-group stationaries, split into K<=24 sub-matmuls
accumulating in PSUM (the PE quarter-row-group path is ~3x faster than
K>=32). The rank-1 grids gd = 1 - W4(y)*zs^4 and x = y*(zs-1) come from
tensor_scalar ops with per-partition scalar vectors instead of matmuls.
DVE/ACT/GPSIMD run the short sqrt chain; one f32r matmul with an
all-ones per-group selector reduces all three integrals for all 1024 zs
at once (f32r is safe here: the |element|-mass to |V| amplification is
<= 3, so TF32-level element rounding stays ~1.5e-3); the tiny tail does
the Vc+Vd combine and the shift.

Numerics: the Vd y-weight mismatch (w/sqrt(y) vs w*y*W2) is folded into
the Pt'/gdd'' stationary coefficients (Pt' = Pt*ratio, gdd'' =
gdd'/ratio, ratio = 1/(y^1.5 W2)), so one selector weight serves all
three chunks. Cancellation-free forms: t1 rows vanish as y->0; 1-zd^4 =
(1-zs)*y2*(1+zd+zd^2+zd^3) with the exact (1-zs)*y2 factor folded into
weights/scales. Pt = fzd*gnd as a single polynomial has ~45x coefficient
amplification at zd->0.1, which is why the grid matmuls must be true
fp32, not f32r (TF32-ish): f32r grids fail the 2e-2 gate at small zs.
"""

import math
import numpy as np

B_TOTAL = 8192
NCORES = 8
BPC = B_TOTAL // NCORES          # 1024 zs per core
NY = 16                          # Gauss-Legendre nodes
G = 8                            # zs groups per core
JC = BPC // G                    # 128 zs per group (free dim)

# cstA (connected): rhsC | lhsTC -- contiguous DRAM param
RC0 = 0            # rhsC [48 rows, 128]
LC0 = 128          # lhsTC [48 rows, 3*128]  (t1 | fz | gn)
CAW = 512
# cstB1 (disconnected): rhsD | gdd lhsT; cstB2: Pt lhsT
RD0 = 0            # rhsD [72 rows, 128]
LD0 = 128          # gdd'' lhsT [72 rows, 128]
CB1W = 256
CB2W = 128

# ckA: per-partition columns + reduce selector (tiny)
W4C = 0            # -W4(y) per-partition column
EC = 1             # y(p) per-partition column
SEL0 = 2           # selector [128, 8]
CKAW = 16
# ckC [8 partitions, 1024]: selT (wL outer-product stationary) | c-rows |
# shift row | ones-selector | z4 group-rows | u group-rows
SLT0 = 0           # selT [8, 128]: wL(y) per (g,y) column, row g
CR0 = 128          # c-rows [8, 3*128]
CT0 = 512          # shift row [8, 128]
ON80 = 640         # ones selector [8, 128]
Z4R0 = 768         # zs^4 rows [8, 128]
UR0 = 896          # (zs-1) rows [8, 128]
CKCW = 1024

_COMPILED = {}
SPLIT_MM = False


def _build_host_tables(a, b, logcoef, shift, zs):
    """All derived constants in float64, cast to f32 at the end."""
    a = np.asarray(a, np.float64)
    b = np.asarray(b, np.float64)
    lc = float(np.asarray(logcoef).reshape(-1)[0])
    sh = float(np.asarray(shift).reshape(-1)[0])
    zs = np.asarray(zs, np.float64)

    t, wq = np.polynomial.legendre.leggauss(NY)
    y = 0.5 * (t + 1.0)
    wq = 0.5 * wq                         # nodes/weights on [0,1]

    fa1 = 4.0 / 3.0 * a[0]
    fa2 = 2.0 * a[1]
    fa4 = -(1.0 + fa1 + fa2)

    w1 = 1.0 - y * y
    W2 = w1 * w1
    W4 = W2 * W2
    e = y
    ratio = 1.0 / (y ** 1.5 * W2)         # Vd-weight / LVc-weight
    wL = wq * y * W2                      # the single selector weight
    ones = np.ones(NY)

    # connected kinds, 32-aligned blocks: rows 0:24 = {1, z, z2},
    # rows 32:48 = {z4, fs}
    # kind indices: 0='1', 1='z', 2='z2' in block0; 4='z4', 5='fs' in
    # block1 (rows 32:40, 40:48)
    gn_c = {0: ones, 1: b[0] * w1, 2: b[1] * W2}
    fz_c = {0: ones, 1: fa1 * w1, 2: fa2 * W2, 4: fa4 * W4}
    t1_c = {1: fa1 * (w1 - 1), 2: fa2 * (W2 - 1), 4: fa4 * (W4 - 1),
            5: 1.0 - W4}

    # disconnected kinds: rows 0:24 = {1, u, u2}, rows 32:56 = {u3,u4,u5},
    # rows 64:72 = {u6}
    g1 = fa1 + 2 * fa2 + 4 * fa4
    g2 = fa2 + 6 * fa4
    g3 = 4 * fa4
    g4 = fa4
    d0 = 1.0 + b[0] + b[1]
    d1 = b[0] + 2 * b[1]
    d2 = b[1]
    q = np.convolve([0.0, g1, g2, g3, g4], [d0, d1, d2])   # fzd*gnd, powers 0..6

    pt_c = {1: q[1] * e * ratio, 2: q[2] * e**2 * ratio,
            3: q[3] * e**3 * ratio, 4: q[4] * e**4 * ratio,
            5: q[5] * e**5 * ratio, 6: q[6] * e**6 * ratio}
    gd_c = {0: 4 * ones / ratio, 1: 6 * e / ratio, 2: 4 * e**2 / ratio,
            3: e**3 / ratio}

    # kind index -> (row offset) maps
    crow = {0: 0, 1: 8, 2: 16, 4: 32, 5: 40}           # connected, 8 rows/kind
    drow = {0: 0, 1: 8, 2: 16, 3: 32, 4: 40, 5: 48, 6: 64}  # disconnected

    def blockdiag(coefs, rowmap, nrows):
        out = np.zeros((nrows, G * NY))
        for k, cy in coefs.items():
            r0 = rowmap[k]
            for g in range(G):
                out[r0 + g, g * NY:(g + 1) * NY] = cy
        return out

    lhsTC = np.zeros((48, 3 * JC))
    lhsTC[:, 0:JC] = blockdiag(t1_c, crow, 48)
    lhsTC[:, JC:2 * JC] = blockdiag(fz_c, crow, 48)
    lhsTC[:, 2 * JC:3 * JC] = blockdiag(gn_c, crow, 48)
    lhsTD = np.zeros((72, 2 * JC))
    lhsTD[:, 0:JC] = blockdiag(gd_c, drow, 72)
    lhsTD[:, JC:2 * JC] = blockdiag(pt_c, drow, 72)

    ckA_shared = np.zeros((G * NY, CKAW))
    selT_shared = np.zeros((G, G * NY))
    one8_shared = np.zeros((G, G * NY))
    for g in range(G):
        ckA_shared[g * NY:(g + 1) * NY, SEL0 + g] = 1.0
        ckA_shared[g * NY:(g + 1) * NY, W4C] = -W4
        ckA_shared[g * NY:(g + 1) * NY, EC] = e
        selT_shared[g, g * NY:(g + 1) * NY] = wL
        one8_shared[g, g * NY:(g + 1) * NY] = 1.0

    # ---- per-core zs-derived tables ----
    cstA_all, cstB1_all, cstB2_all, ckC_all = [], [], [], []
    elc = math.exp(lc)
    for c in range(NCORES):
        z = zs[c * BPC:(c + 1) * BPC]
        z2 = z * z
        z4 = z2 * z2
        fs = 1.0 + fa1 * z + fa2 * z2 + fa4 * z4
        u = z - 1.0
        u2 = u * u
        u3 = u2 * u

        def rows(kinds, rowmap, nrows):
            out = np.zeros((nrows, JC))
            for k, kv in kinds.items():
                r0 = rowmap[k]
                out[r0:r0 + G, :] = kv.reshape(G, JC)
            return out

        cstA = np.zeros((48, CAW))
        cstA[0:48, RC0:RC0 + JC] = rows(
            {0: np.ones(BPC), 1: z, 2: z2, 4: z4, 5: fs}, crow, 48)
        cstA[0:48, LC0:LC0 + 3 * JC] = lhsTC
        cstB1 = np.zeros((72, CB1W))
        cstB1[0:72, RD0:RD0 + JC] = rows(
            {0: np.ones(BPC), 1: u, 2: u2, 3: u3, 4: u2 * u2, 5: u2 * u3,
             6: u3 * u3}, drow, 72)
        cstB1[0:72, LD0:LD0 + JC] = lhsTD[:, 0:JC]
        cstB2 = lhsTD[:, JC:2 * JC].copy()

        ckC = np.zeros((G, CKCW))
        ckC[:, SLT0:SLT0 + 128] = selT_shared
        ckC[:, ON80:ON80 + 128] = one8_shared
        ckC[:, CR0 + 0 * JC:CR0 + 1 * JC] = (
            4.0 * z * np.sqrt(fs) / math.pi).reshape(G, JC)
        ckC[:, CR0 + 1 * JC:CR0 + 2 * JC] = (
            4.0 * math.pi * fs * elc / z).reshape(G, JC)
        ckC[:, CR0 + 2 * JC:CR0 + 3 * JC] = (
            -2.0 * math.pi * np.sqrt(1.0 - z) * elc).reshape(G, JC)
        ckC[:, CT0:CT0 + JC] = sh
        ckC[:, Z4R0:Z4R0 + JC] = z4.reshape(G, JC)
        ckC[:, UR0:UR0 + JC] = u.reshape(G, JC)

        cstA_all.append(cstA.astype(np.float32))
        cstB1_all.append(cstB1.astype(np.float32))
        cstB2_all.append(cstB2.astype(np.float32))
        ckC_all.append(ckC.astype(np.float32))

    return cstA_all, cstB1_all, cstB2_all, ckA_shared.astype(np.float32), \
        ckC_all


def _patch_tile_drain():
    """Walrus rejects instructions with >4 sync waits; Tile's kernel-tail
    drain waits on every active processor at once. Split it into one drain
    per processor (SP-engine drains are ~12 ns each)."""
    import re as _re
    import concourse.tile as tile_mod
    import bass_rust
    from bass_rust import ScopedClock

    if getattr(tile_mod.TileContext, "_drain_patched", False):
        return

    def _patched(self, tick_clock, wait_clock):
        gc = tick_clock.global_clock
        ticks = [int(x) for x in _re.findall(r"\d+", repr(gc))]
        for i in [i for i, t in enumerate(ticks) if t > 0]:
            sub = bass_rust.VectorClock()
            sub.require_at_least(i, ticks[i])
            d = self.nc.sync.drain()
            wait_clock.add_sem_waits(d.ins, ScopedClock({None: sub}))
        self.nc.all_engine_barrier()
        popped = self.nc._tile_sem_poison_stack.pop()
        assert popped is self._sem_poison
        self.nc.clear_and_free_semaphores(list(self.sems.allocated().values()))

    tile_mod.TileContext._drain_and_barrier = _patched
    tile_mod.TileContext._drain_patched = True


def _prune_redundant_waits(nc):
    """Tile emits per-instruction sem waits that are not transitively minimal
    (syncing on engine X does not teach it what X itself had waited on), but
    every TPB instruction has exactly ONE sync-wait slot. Run a vector-clock
    closure over the scheduled program, drop every wait already implied by
    the instruction's processor, and hoist any excess waits onto earlier
    same-processor instructions with a free slot (cycle-checked)."""
    insts = []
    for blk in nc.m.functions[0].blocks:
        insts.extend(blk.instructions)

    nonmono = set()
    for inst in insts:
        si = inst.sync_info
        if si is None:
            continue
        for u in si.on_update or []:
            nm = getattr(u, "ant_name", "") or ""
            if getattr(u, "sync_type", "") == "semaphore" and \
                    getattr(u, "update_mode", "") != "sem-inc" and \
                    "barrier" in nm:
                nonmono.add(u.id)
        for w in si.on_wait or []:
            nm = getattr(w, "ant_name", "") or ""
            if "barrier" in nm:
                nonmono.add(w.id)

    V = {}
    snap = {}
    cnt = {}
    own_sem = {}
    free_slots = {}

    def proc_key(inst):
        si = inst.sync_info
        if si is not None:
            for u in si.on_update or []:
                nm = getattr(u, "ant_name", "") or ""
                if nm.startswith("DMA"):
                    return nm
        return str(inst.engine)

    def dep_state(sem, val):
        snaps = snap.get(sem)
        if not snaps:
            return None
        keys = [k for k in snaps if k >= val]
        if not keys:
            return None
        return snaps[min(keys)]

    def merge_from(state, sem, val):
        state[sem] = max(state.get(sem, 0), val)
        ds = dep_state(sem, val)
        if ds:
            for s2, v2 in ds.items():
                if state.get(s2, 0) < v2:
                    state[s2] = v2

    n_dropped = n_hoisted = n_left = 0
    for inst in insts:
        si = inst.sync_info
        pk = proc_key(inst)
        state = V.setdefault(pk, {})
        my_sem = own_sem.get(pk)
        if si is not None and si.on_wait:
            kept = []
            movable = []
            dropped_here = set()
            prestate = dict(state)
            for w in si.on_wait:
                if getattr(w, "sync_type", "") != "semaphore" or \
                        getattr(w, "wait_mode", "") != "sem-ge-imm" or \
                        w.id in nonmono:
                    kept.append(w)
                    continue
                sem, val = w.id, w.wait_value
                # droppable if implied by the processor's prior state or by
                # the transitive closure of the other KEPT waits on this inst
                others = dict(prestate)
                for w2 in si.on_wait:
                    if w2 is w or getattr(w2, "sync_type", "") != "semaphore" \
                            or getattr(w2, "wait_mode", "") != "sem-ge-imm" \
                            or w2.id in nonmono or id(w2) in dropped_here:
                        continue
                    merge_from(others, w2.id, w2.wait_value)
                if others.get(sem, 0) >= val:
                    n_dropped += 1
                    dropped_here.add(id(w))
                else:
                    movable.append(w)
                merge_from(state, sem, val)
            while len(kept) + len(movable) > 1 and movable:
                w = movable.pop(0)
                placed = False
                for tsi, ttick in reversed(free_slots.get(pk, [])):
                    ds = dep_state(w.id, w.wait_value) or {}
                    if my_sem is not None and ds.get(my_sem, 0) >= ttick:
                        continue
                    if not ds:
                        continue
                    tsi.on_wait = [w]
                    free_slots[pk].remove((tsi, ttick))
                    placed = True
                    n_hoisted += 1
                    break
                if not placed:
                    kept.append(w)
                    n_left += 1
            kept.extend(movable)
            if len(kept) != len(si.on_wait):
                si.on_wait = kept
        if si is not None:
            for u in si.on_update or []:
                if getattr(u, "sync_type", "") != "semaphore":
                    continue
                sem = u.id
                if getattr(u, "update_mode", "") != "sem-inc" or sem in nonmono:
                    continue
                uv = getattr(u, "update_value", 1) or 1
                cnt[sem] = cnt.get(sem, 0) + uv
                if not pk.startswith("DMA"):
                    own_sem.setdefault(pk, sem)
                here = dict(state)
                here[sem] = cnt[sem]
                snap.setdefault(sem, {})[cnt[sem]] = here
                state[sem] = cnt[sem]
        if (si is not None and not si.on_wait and not pk.startswith("DMA")
                and str(getattr(inst, "opcode", "")) not in
                ("Matmult", "EventSemaphore", "Drain",
                 "EventSemaphoreRangeClear", "UnconditionalBranch",
                 "CompareBranch", "SetOrderingMode", "Move", "Notify", "Nop")
                and "barrier" not in (inst.name or "")):
            free_slots.setdefault(pk, []).append(
                (si, cnt.get(own_sem.get(pk, -1), 0)))
    if n_left:
        import logging
        logging.warning("_prune_redundant_waits: %d waits could not be "
                        "hoisted; compile may fail", n_left)
    return n_dropped, n_hoisted, n_left


def _act_raw(nc, mybir, func, out, in_, scale=1.0, bias=0.0):
    eng = nc.scalar
    return eng.add_instruction(mybir.InstActivation(
        name=nc.get_next_instruction_name(), func=func,
        ins=[eng.lower_ap(in_),
             mybir.ImmediateValue(dtype=mybir.dt.float32, value=bias),
             mybir.ImmediateValue(dtype=mybir.dt.float32, value=scale),
             mybir.ImmediateValue(dtype=mybir.dt.float32, value=0.0)],
        outs=[eng.lower_ap(out)]))


def _build_nc(prune=True):
    import concourse.bass as bass
    import concourse.mybir as mybir
    from concourse.tile import TileContext
    from concourse.bass import _add_dep_helper

    f32 = mybir.dt.float32
    f32r = mybir.dt.float32r
    AF = mybir.ActivationFunctionType
    ALU = mybir.AluOpType

    _patch_tile_drain()
    nc = bass.Bass(enable_partition_id=False)
    cstA_d = nc.declare_dram_parameter("cstA", [48, CAW], f32, isOutput=False)
    cstB1_d = nc.declare_dram_parameter("cstB1", [72, CB1W], f32,
                                        isOutput=False)
    cstB2_d = nc.declare_dram_parameter("cstB2", [72, CB2W], f32,
                                        isOutput=False)
    ckA_d = nc.declare_dram_parameter("ckA", [128, CKAW], f32, isOutput=False)
    ckC_d = nc.declare_dram_parameter("ckC", [G, CKCW], f32, isOutput=False)
    out_d = nc.declare_dram_parameter("out", [G, 2 * JC], f32, isOutput=True)

    with TileContext(nc) as tc:
        with (
            tc.tile_pool(name="const", bufs=1) as cp,
            tc.tile_pool(name="work", bufs=1) as wp,
            tc.tile_pool(name="ps", bufs=1, space="PSUM") as pp,
        ):
            # ---- constants: five small contiguous DMAs on two rings ----
            kc = cp.tile([G, CKCW], f32r)
            nc.scalar.dma_start(out=kc[:], in_=ckC_d[:].bitcast(f32r))
            ka = cp.tile([128, CKAW], f32)
            nc.scalar.dma_start(out=ka[:], in_=ckA_d[:])
            ca = cp.tile([48, CAW], f32r)
            nc.sync.dma_start(out=ca[:], in_=cstA_d[:].bitcast(f32r))
            cb1 = cp.tile([72, CB1W], f32r)
            nc.sync.dma_start(out=cb1[:], in_=cstB1_d[:].bitcast(f32r))
            cb2 = cp.tile([72, CB2W], f32)
            nc.sync.dma_start(out=cb2[:], in_=cstB2_d[:])

            def rc(r0, r1):
                return ca[r0:r1, RC0:RC0 + JC]

            def rd(r0, r1):
                return cb1[r0:r1, RD0:RD0 + JC]

            def lc(i, r0, r1):
                return ca[r0:r1, LC0 + i * JC:LC0 + (i + 1) * JC]

            # ---- polynomial grids via PE.  t1/fz/gn/gdd and the two
            # replication matmuls are f32r (coefficient amplification <= 5,
            # TF32 rounding ~2e-3 is fine); Pt has ~45x amplification and
            # must be true fp32.  Single matmul per grid: fp32 multi-matmul
            # PSUM accumulation hangs the HW.  One PSUM tile per grid. ----
            # PSUM is 8 banks x 2KB: pack grids into banks so pairs never
            # stall their consumers (left grid's matmul runs first)
            T1 = pp.tile([128, JC], f32, tag="T1")
            GF = pp.tile([128, 256], f32, tag="GF")    # [gn | fz]
            GDD = pp.tile([128, JC], f32, tag="GDD")
            PS = pp.tile([128, 512], f32, tag="PS")    # [SCLp | Pt]
            ZU = pp.tile([128, 256], f32, tag="ZU")    # [z4rep | urep]
            Fp = pp.tile([G, 3 * JC], f32, tag="F")    # reduce output
            GN = GF[:, 0:128]
            FZ = GF[:, 128:256]
            SCLP = PS[:, 0:3 * JC]
            PT = PS[:, 3 * JC:4 * JC]
            Z4P = ZU[:, 0:128]
            URP = ZU[:, 128:256]

            MM = dict(skip_group_check=True)
            one8 = kc[0:G, ON80:ON80 + 128]
            nc.tensor.matmul(Z4P, one8, kc[0:G, Z4R0:Z4R0 + JC],
                             start=True, stop=True, **MM)
            nc.tensor.matmul(URP, one8, kc[0:G, UR0:UR0 + JC],
                             start=True, stop=True, **MM)
            nc.tensor.matmul(SCLP, kc[0:G, SLT0:SLT0 + 128],
                             kc[0:G, CR0:CR0 + 3 * JC],
                             start=True, stop=True, **MM)
            # ---- rank-1 grids (per-partition scalars on replicated rows) ----
            tvk = cp.tile([1, 1], f32)
            d0 = nc.vector.tensor_copy(out=tvk[:], in_=ka[0:1, 0:1])
            tvk2 = cp.tile([1, 1], f32)
            d0b = nc.vector.tensor_copy(out=tvk2[:],
                                        in_=kc[0:1, 0:1].bitcast(f32))
            GD = wp.tile([128, JC], f32, tag="GD")      # 1 - W4*zs^4
            d1 = nc.vector.tensor_scalar(out=GD[:], in0=Z4P,
                                         scalar1=ka[:, W4C:W4C + 1],
                                         scalar2=1.0, op0=ALU.mult,
                                         op1=ALU.add)
            XD = wp.tile([128, JC], f32, tag="XD")      # x = y*(zs-1)
            d2 = nc.vector.tensor_scalar(out=XD[:], in0=URP,
                                         scalar1=ka[:, EC:EC + 1],
                                         scalar2=None, op0=ALU.mult)
            # the f32r reduce needs an f32r-declared stationary
            selr = wp.tile([128, G], f32r, tag="selr")
            d3 = nc.vector.tensor_copy(out=selr[:], in_=ka[:, SEL0:SEL0 + G])

            nc.tensor.matmul(GN, lc(2, 0, 24), rc(0, 24),
                             start=True, stop=True, **MM)
            nc.tensor.matmul(T1[:], lc(0, 0, 48), rc(0, 48),
                             start=True, stop=True, **MM)
            nc.tensor.matmul(FZ, lc(1, 0, 40), rc(0, 40),
                             start=True, stop=True, **MM)
            nc.tensor.matmul(GDD[:], cb1[0:40, LD0:LD0 + JC], rd(0, 40),
                             start=True, stop=True, **MM)
            nc.tensor.matmul(PT, cb2[0:72, :],
                             rd(0, 72).bitcast(f32),
                             start=True, stop=True, **MM)

            # ---- elementwise chains; emission order = intended schedule,
            # dep chains pin walrus's static order.  Tiny per-engine
            # absorbers of the T1 matmul keep ACT and DVE independent
            # (Tile otherwise piggybacks one engine's PE dep through the
            # other, serializing the two chain heads). ----
            Z2S = wp.tile([128, JC], f32, tag="Z2S")
            a4 = _act_raw(nc, mybir, AF.Square, Z2S[:], XD[:],
                          scale=1.0, bias=1.0)
            Z4S = wp.tile([128, JC], f32, tag="Z4S")
            a5 = _act_raw(nc, mybir, AF.Square, Z4S[:], Z2S[:])
            tw1 = cp.tile([1, 1], f32)
            a0 = nc.scalar.copy(out=tw1[:], in_=T1[0:1, 0:1])
            tw2 = cp.tile([1, 1], f32)
            d3b = nc.vector.tensor_copy(out=tw2[:], in_=T1[0:1, 0:1])
            GT = wp.tile([128, JC], f32, tag="GT")      # gd*t1
            d4 = nc.vector.tensor_mul(GT[:], T1[:], GD[:])
            T1S = wp.tile([128, JC], f32, tag="T1S")
            a1 = nc.scalar.copy(out=T1S[:], in_=T1[:])
            MW = wp.tile([128, 256], f32, tag="MW")     # [t1*fz | gn*gd*t1]
            d5 = nc.vector.tensor_mul(MW[:, 128:256], GN, GT[:])
            d6 = nc.vector.tensor_mul(MW[:, 0:128], FZ, T1S[:])
            RQW = wp.tile([128, 256], f32, tag="RQW")
            a2 = _act_raw(nc, mybir, AF.Rsqrt, RQW[:, 128:256],
                          MW[:, 128:256])
            a3 = _act_raw(nc, mybir, AF.Rsqrt, RQW[:, 0:128], MW[:, 0:128])

            RRST = wp.tile([128, 3 * JC], f32, tag="RRST")
            d7 = nc.vector.tensor_mul(RRST[:, 0:128], GN, RQW[:, 128:256])
            ST = wp.tile([128, JC], f32, tag="ST")
            p1 = nc.gpsimd.tensor_mul(ST[:], T1S[:], RQW[:, 0:128])
            USQ = wp.tile([128, JC], f32, tag="USQ")
            a6 = _act_raw(nc, mybir, AF.Square, USQ[:], ST[:],
                          scale=1.0, bias=1.0)
            RDEN = wp.tile([128, JC], f32, tag="RDEN")
            a7 = _act_raw(nc, mybir, AF.Rsqrt, RDEN[:], USQ[:])
            tvd = cp.tile([1, 1], f32)
            p1b = nc.gpsimd.tensor_copy(out=tvd[:], in_=RRST[0:1, 0:1])
            p2 = nc.gpsimd.tensor_mul(RRST[:, 128:256], RRST[:, 0:128],
                                      RDEN[:])

            G1 = wp.tile([128, JC], f32, tag="G1")
            d8 = nc.vector.tensor_mul(G1[:], GDD[:], Z4S[:])
            PG = wp.tile([128, JC], f32, tag="PG")
            d9 = nc.vector.tensor_mul(PG[:], PT, G1[:])
            R2 = wp.tile([128, JC], f32, tag="R2")
            a8 = _act_raw(nc, mybir, AF.Rsqrt, R2[:], PG[:])
            d10 = nc.vector.tensor_mul(RRST[:, 256:384], PT, R2[:])

            # ---- fold weights+scales, then ONE f32r reduce matmul ----
            RRS = wp.tile([128, 3 * JC], f32r, tag="RRS")
            d11 = nc.vector.tensor_mul(RRS[:], RRST[:], SCLP)
            # reduce in two matmuls: the Vd chunk ACCUMULATES onto the Vc
            # sums in PSUM, so the tail needs no Vc+Vd add
            nc.tensor.matmul(Fp[:, 0:2 * JC], selr[:], RRS[:, 0:2 * JC],
                             start=True, stop=False, **MM)
            nc.tensor.matmul(Fp[:, JC:2 * JC], selr[:], RRS[:, 2 * JC:3 * JC],
                             start=False, stop=True, **MM)

            # ---- tail: shift, L copy, one out DMA ----
            FS = wp.tile([G, 2 * JC], f32, tag="FS")
            d12 = nc.vector.tensor_add(FS[:, JC:2 * JC], Fp[:, JC:2 * JC],
                                       kc[0:G, CT0:CT0 + JC].bitcast(f32))
            d13 = nc.vector.tensor_copy(out=FS[:, 0:JC], in_=Fp[:, 0:JC])
            nc.sync.dma_start(out=out_d[:], in_=FS[:])

            for chain in ([a4, a5, a0, a1, a2, a3, a6, a7, a8],
                          [d0, d0b, d1, d2, d3, d3b, d4, d5, d6, d7, d8, d9, d10,
                           d11, d12, d13],
                          [p1, p2]):
                for prev, nxt in zip(chain, chain[1:]):
                    _add_dep_helper(nxt.ins, prev.ins, sync=False,
                                    reason="pin static engine order")

    if prune:
        _prune_redundant_waits(nc)
    return nc


def _get_nc():
    if "nc" not in _COMPILED:
        _COMPILED["nc"] = _build_nc()
    return _COMPILED["nc"]


def kernel(a, b, logcoef, shift, zs, _trace=False):
    from concourse.bass_utils import run_bass_kernel_spmd

    a = np.asarray(a)
    b = np.asarray(b)
    zs = np.asarray(zs)
    assert zs.shape == (B_TOTAL,)

    cstA_all, cstB1_all, cstB2_all, ckA_shared, ckC_all = \
        _build_host_tables(a, b, logcoef, shift, zs)

    in_maps = [
        {"cstA": cstA_all[c], "cstB1": cstB1_all[c], "cstB2": cstB2_all[c],
         "ckA": ckA_shared, "ckC": ckC_all[c]}
        for c in range(NCORES)
    ]

    nc = _get_nc()
    res = run_bass_kernel_spmd(nc, in_maps, core_ids=list(range(NCORES)),
                               trace=_trace)
    # out [G, 2*JC]: cols 0:128 = L, 128:256 = V, per group g
    outs = []
    for c in range(NCORES):
        o = res.results[c]["out"]
        outs.append(np.stack([o[:, 0:JC].reshape(BPC),
                              o[:, JC:2 * JC].reshape(BPC)]))
    out = np.concatenate(outs, axis=1)
    if _trace:
        kernel.last_exec_time_ns = res.exec_time_ns
        kernel.last_profile = res.profile_json
    return out.astype(np.float32)
